# revision 30
# baseline (speedup 1.0000x reference)
"""MultiHeadAttention with relative bias + key padding mask on 8 trn2 NeuronCores.

Sharding: head-parallel — core c owns head pair {2c, 2c+1} for BOTH batches.
Each core computes its heads' attention and a partial o-projection over the
full output dim; the host sums the 8 partials and adds bo_eff.

Device-side formulation (per core, per batch b, per head h):
  qT = (Wq_h/8) @ query_b^T + bq/8     [64, S]  (1/sqrt(DH) folded into Wq,bq)
  kT =  Wk_h    @ key_b^T              [64, S]  (bk dropped: cancels in softmax)
  v  = value_b @ Wv_h^T  in [s, dh] layout, masked rows zeroed, mask-column
       appended (denominator trick); bv folded into bo on the host.
  scoresT[kk,qq] = kT-slice^T . qT-slice              (PE, one N=1024 matmul)
  PT = exp(scoresT) ⊙ exp(biasT)       (ACT exp -> bf16, DVE 2x bf16 multiply;
                                        exp(bias) precomputed on the host)
  attnT[dh,qq] (+ denom row via mask column in v) = v_aug^T @ PT  (N=1024)
  au[h0 dims -> partitions 0:64, h1 -> 64:128] = attnT * recip(denom)
  oT_partial[do,qq] = woT_packed[:,do]^T @ au          (K=128, one matmul/chunk)

Schedule: software pipeline at kk granularity — pass N's score stream is
interleaved with pass N-1's av matmuls and o-projection chunks so the PE
stays fed while ACT exps throttle the score psum ring. Projections run as
solid PE blocks between windows (qk(b1) after w0/w1, v(b1) after w2), each
overlapping the other engines' steady-state work. Bias tiles live in two
SBUF-resident arrays (h0 / h1), each loaded twice (qq=0 then qq=1 refill).

Fully-masked (b, kk) tiles are skipped at program-build time (the program is
cached keyed on the observed mask tile pattern).
"""
import sys

sys.path.insert(0, "/opt/trn_rl_repo")
import numpy as np
import ml_dtypes

import concourse.bass as bass
from concourse import bacc
import concourse.tile as tile
from concourse import mybir
from concourse.bass_utils import run_bass_kernel_spmd

B, S, D, H, DH = 2, 2048, 1024, 16, 64
NC = 8
HPC = H // NC  # heads per core = 2
f32 = mybir.dt.float32
bf16 = mybir.dt.bfloat16
Exp = mybir.ActivationFunctionType.Exp
NK = S // 128  # 16 k-tiles of 128
ND = D // 128  # 8 chunks of the model dim

_PROGRAMS = {}  # keyed by mask tile pattern
_LAST_IN_MAPS = None
_LAST_KEY = None


def _build_program(full_tiles, part_tiles):
    """full_tiles: frozenset of fully-masked (b, kk); part_tiles: frozenset of
    partially-masked (b, kk) needing per-tile v-row zeroing."""
    nc = bacc.Bacc(None, target_bir_lowering=False)
    d = {}
    d["queryT"] = nc.declare_dram_parameter("queryT", [B, D, S], bf16, isOutput=False)
    d["keyT"] = nc.declare_dram_parameter("keyT", [B, D, S], bf16, isOutput=False)
    d["valueT"] = nc.declare_dram_parameter("valueT", [B, D, S], bf16, isOutput=False)
    d["ebiasT"] = nc.declare_dram_parameter("ebiasT", [HPC, S, S], bf16, isOutput=False)
    d["vcol"] = nc.declare_dram_parameter("vcol", [128, B, NK, 1], bf16, isOutput=False)
    d["vmask"] = nc.declare_dram_parameter("vmask", [128, B, NK], f32, isOutput=False)
    d["wqT"] = nc.declare_dram_parameter("wqT", [128, ND * 128], bf16, isOutput=False)
    d["wkT"] = nc.declare_dram_parameter("wkT", [128, ND * 128], bf16, isOutput=False)
    d["wvT"] = nc.declare_dram_parameter("wvT", [128, ND * 128], bf16, isOutput=False)
    d["bq"] = nc.declare_dram_parameter("bq", [128, 1], f32, isOutput=False)
    d["woT"] = nc.declare_dram_parameter("woT", [128, D], bf16, isOutput=False)
    oT = nc.declare_dram_parameter("oT", [B, D, S], bf16, isOutput=True)

    live_kk = {b: [kk for kk in range(NK) if (b, kk) not in full_tiles]
               for b in range(B)}

    with tile.TileContext(nc) as tc:
        with (
            tc.tile_pool(name="const", bufs=1) as const,
            tc.tile_pool(name="persist", bufs=1) as persist,
            tc.tile_pool(name="xt", bufs=8) as xt,
            tc.tile_pool(name="etp", bufs=10) as etp,
            tc.tile_pool(name="ptw", bufs=16) as ptw,
            tc.tile_pool(name="otp", bufs=3) as otp,
            tc.tile_pool(name="rrp", bufs=1) as rrp,
            tc.tile_pool(name="bcp", bufs=1) as bcp,
            tc.tile_pool(name="psS", bufs=2, space="PSUM") as psS,
            tc.tile_pool(name="psX", bufs=1, space="PSUM") as psX,
            tc.tile_pool(name="psT", bufs=1, space="PSUM") as psT,
        ):
            def mm512(out, lhsT, rhs, start, stop, **kw):
                """Matmul split into 512-col pieces: an f32 psum out must not
                cross a 2KB bank boundary."""
                n = rhs.shape[-1] if hasattr(rhs, "shape") else None
                ncols = out.shape[-1]
                for c in range(0, ncols, 512):
                    w = min(512, ncols - c)
                    nc.tensor.matmul(out=out[..., c : c + w], lhsT=lhsT,
                                     rhs=rhs[..., c : c + w], start=start,
                                     stop=stop, **kw)

            def ptile(pool, name):
                tag = {id(psS): "mm", id(psX): "px", id(psT): "pt"}[id(pool)]
                return pool.tile([128, 1024], f32, tag=tag, name=name)

            w_sb = {}
            for nm in ("wq", "wk", "wv"):
                w_sb[nm] = const.tile([128, ND, 128], bf16, tag=nm, name="w_" + nm)
            bq_sb = const.tile([128, 1], f32, tag="bq", name="bq_sb")
            wo_sb = const.tile([128, D], bf16, tag="wo", name="wo_sb")
            vm_sb = const.tile([128, B, NK], f32, tag="vm", name="vm_sb")
            warm = const.tile([1, 2], f32, tag="warm", name="warm")

            qT_sb = persist.tile([128, B, S], bf16, tag="qT", name="qT_sb")
            kT_sb = persist.tile([128, B, S], bf16, tag="kT", name="kT_sb")
            v_sb = persist.tile([128, B, NK, HPC, 66], bf16, tag="v", name="v_sb")
            # packed attention output: h0 dims on partitions 0:64, h1 on 64:128
            au_sb = persist.tile([128, B, 2, 1024], bf16, tag="au", name="au_sb")

            # two resident bias arrays: A serves h=0 passes, B serves h=1;
            # each holds one qq-column block at a time (reloaded for qq=1)
            btA = persist.tile([128, NK, 1024], bf16, tag="btA", name="btA")
            btB = persist.tile([128, NK, 1024], bf16, tag="btB", name="btB")

            def load_bias(qq, h, arr, eng, gate=None):
                """gate: SBUF AP whose producer must finish first. The gate
                copy writes one element into every kk region of the target
                array, so each tile DMA picks up a WAR dependency on it —
                the x-streams own the DMA device until the gate resolves."""
                if gate is not None:
                    nc.gpsimd.tensor_copy(out=arr[0:1, :, 0:1], in_=gate)
                q0 = qq * 1024
                for kk in range(NK):
                    if any((bb, kk) not in full_tiles for bb in range(B)):
                        eng.dma_start(
                            out=arr[:, kk, :],
                            in_=d["ebiasT"][h, kk * 128 : (kk + 1) * 128,
                                            q0 : q0 + 1024],
                        )

            # ---------------- projections (one-pass blocks) ----------------
            def proj_kq0(ptk0, ptk1, ptq0, ptq1):
                """Fused k/q projection of batch 0: interleaved x streams."""
                ptk, ptq = [ptk0, ptk1], [ptq0, ptq1]
                lastc = (live_kk[0][-1] + 1) * 128
                for dc in range(ND):
                    xck = xt.tile([128, S], bf16, tag="xc", name="xck")
                    nc.sync.dma_start(
                        out=xck[:], in_=d["keyT"][0, dc * 128 : (dc + 1) * 128, :]
                    )
                    xcq = xt.tile([128, S], bf16, tag="xc", name="xcq")
                    nc.sync.dma_start(
                        out=xcq[:], in_=d["queryT"][0, dc * 128 : (dc + 1) * 128, :]
                    )
                    for qh in range(2):
                        c0 = qh * 1024
                        c1 = min(c0 + 1024, lastc)
                        if c1 > c0:
                            mm512(ptk[qh][:, 0 : c1 - c0],
                                  w_sb["wk"][:, dc, :], xck[:, c0:c1],
                                  start=(dc == 0), stop=(dc == ND - 1))
                        mm512(ptq[qh][:], w_sb["wq"][:, dc, :],
                              xcq[:, qh * 1024 : (qh + 1) * 1024],
                              start=(dc == 0), stop=(dc == ND - 1))
                # qh0 copies first: w0 (qq=0, kk<8) starts immediately
                nc.scalar.copy(out=kT_sb[:, 0, 0:1024], in_=ptk[0][:])
                nc.vector.tensor_scalar_add(
                    out=qT_sb[:, 0, 0:1024], in0=ptq[0][:], scalar1=bq_sb[:]
                )
                nc.vector.tensor_copy(out=kT_sb[:, 0, 1024:2048], in_=ptk[1][:])
                nc.vector.tensor_scalar_add(
                    out=qT_sb[:, 0, 1024:2048], in0=ptq[1][:], scalar1=bq_sb[:]
                )

            def proj_k1(pt0, pt1):
                pt = [pt0, pt1]
                lastc = (live_kk[1][-1] + 1) * 128
                for dc in range(ND):
                    xc = xt.tile([128, S], bf16, tag="xc", name="xck1")
                    nc.sync.dma_start(
                        out=xc[:], in_=d["keyT"][1, dc * 128 : (dc + 1) * 128, :]
                    )
                    for qh in range(2):
                        c0 = qh * 1024
                        c1 = min(c0 + 1024, lastc)
                        if c1 <= c0:
                            continue
                        mm512(pt[qh][:, 0 : c1 - c0], w_sb["wk"][:, dc, :],
                              xc[:, c0:c1],
                              start=(dc == 0), stop=(dc == ND - 1))
                nc.scalar.copy(out=kT_sb[:, 1, 0:1024], in_=pt[0][:])
                lastc1 = min(2048, lastc)
                if lastc1 > 1024:
                    nc.vector.tensor_copy(
                        out=kT_sb[:, 1, 1024:lastc1], in_=pt[1][:, 0 : lastc1 - 1024]
                    )

            def proj_q1(pt0, pt1):
                pt = [pt0, pt1]
                for dc in range(ND):
                    xc = xt.tile([128, S], bf16, tag="xc", name="xcq1")
                    nc.sync.dma_start(
                        out=xc[:], in_=d["queryT"][1, dc * 128 : (dc + 1) * 128, :]
                    )
                    for qh in range(2):
                        mm512(pt[qh][:], w_sb["wq"][:, dc, :],
                              xc[:, qh * 1024 : (qh + 1) * 1024],
                              start=(dc == 0), stop=(dc == ND - 1))
                nc.scalar.add(out=qT_sb[:, 1, 0:1024], in_=pt[0][:], add=bq_sb[:])
                nc.vector.tensor_scalar_add(
                    out=qT_sb[:, 1, 1024:2048], in0=pt[1][:], scalar1=bq_sb[:]
                )

            def v_copy_half(b, pv, half):
                """Copy half the v projection psum into v_sb (bf16), zeroing
                masked key rows where needed."""
                sts = [st for st in range(half * 8, (half + 1) * 8)]
                simple = [st for st in sts
                          if (b, st) not in part_tiles and (b, st) not in full_tiles]
                run = []
                for st in sts:
                    if st in simple and (not run or st == run[-1] + 1):
                        run.append(st)
                    elif not run:
                        continue
                    else:
                        break
                if run:
                    st0, n = run[0], len(run)
                    nc.vector.tensor_copy(
                        out=v_sb[:, b, st0 : st0 + n, :, 0:64],
                        in_=pv[
                            :, (st0 - half * 8) * 128 : (st0 - half * 8 + n) * 128
                        ].rearrange("p (t h m) -> p t h m", t=n, h=HPC),
                    )
                for st in sts:
                    if st in run or (b, st) in full_tiles:
                        continue
                    i0 = (st - half * 8) * 128
                    if (b, st) in part_tiles:
                        nc.vector.tensor_scalar_mul(
                            out=v_sb[:, b, st, :, 0:64],
                            in0=pv[:, i0 : i0 + 128].rearrange(
                                "p (h m) -> p h m", h=HPC
                            ),
                            scalar1=vm_sb[:, b, st : st + 1],
                        )
                    else:
                        nc.vector.tensor_copy(
                            out=v_sb[:, b, st, :, 0:64],
                            in_=pv[:, i0 : i0 + 128].rearrange(
                                "p (h m) -> p h m", h=HPC
                            ),
                        )

            def proj_v(b, pvA, pvB):
                """One-pass v projection: both psum halves live, x JIT."""
                psv = [pvA, pvB]
                for dc in range(ND):
                    xc = xt.tile([128, S], bf16, tag="xc", name=f"xcv{b}")
                    nc.sync.dma_start(
                        out=xc[:], in_=d["valueT"][b, dc * 128 : (dc + 1) * 128, :]
                    )
                    for st in range(NK):
                        # start_tensor_calc zeroes the whole 2KB PSUM bank (4
                        # st-regions): only the bank-first st may set it. Skip
                        # fully-masked st tiles unless needed for bank zeroing.
                        if (b, st) in full_tiles and (
                            st % 4 != 0
                            or all((b, s) in full_tiles
                                   for s in range(st, min(st + 4, NK)))
                        ):
                            continue
                        nc.tensor.matmul(
                            out=psv[st // 8][:, (st % 8) * 128 : (st % 8 + 1) * 128],
                            lhsT=xc[:, st * 128 : (st + 1) * 128],
                            rhs=w_sb["wv"][:, dc, :],
                            start=(dc == 0 and st % 4 == 0),
                            stop=(dc == ND - 1),
                            skip_group_check=True,
                        )
                v_copy_half(b, psv[0], 0)
                v_copy_half(b, psv[1], 1)

            # ---------------- o-projection ----------------
            # po tiles come from the score pool: short exp-like lifetimes
            # keep the psS ring FIFO
            def oproj_chunk(qq, b, do, copy_eng, pool=None):
                q0 = qq * 1024

                def emit():
                    po = ptile(pool if pool is not None else psS,
                               f"po{qq}{b}{do}")
                    mm512(po[:], wo_sb[:, do * 128 : (do + 1) * 128],
                          au_sb[:, b, qq, :], start=True, stop=True)
                    ot = otp.tile([128, 1024], bf16, tag="ot", name="ot")
                    if copy_eng == "act":
                        nc.scalar.copy(out=ot[:], in_=po[:])
                    else:
                        nc.vector.tensor_copy(out=ot[:], in_=po[:])
                    nc.sync.dma_start(
                        out=oT[b, do * 128 : (do + 1) * 128, q0 : q0 + 1024],
                        in_=ot[:],
                    )
                return emit

            # ---------------- attention pass pieces ----------------
            def score_step(qq, h, b, kk, arr, pts, i=0, ring3=None):
                """One kk step: score matmul -> exp -> bias multiply. ring3:
                extra pool rotated in every 3rd step for a 3-deep score ring
                (absorbs cold-PE latency in w0)."""
                q0 = qq * 1024
                if ring3 is not None and i % 3 == 2:
                    sc = ptile(ring3, "scx")
                else:
                    sc = ptile(psS, "sc")
                mm512(sc[:],
                      kT_sb[h * 64 : (h + 1) * 64, b, kk * 128 : (kk + 1) * 128],
                      qT_sb[h * 64 : (h + 1) * 64, b, q0 : q0 + 1024],
                      start=True, stop=True)
                et = etp.tile([128, 1024], bf16, tag="et", name="et")
                nc.scalar.activation(out=et[:], in_=sc[:], func=Exp)
                pt = ptw.tile([128, 1024], bf16, tag="ptw", name="pt")
                nc.vector.tensor_mul(out=pt[:], in0=et[:], in1=arr[:, kk, :])
                pts[kk] = pt

            def av_step(qq, h, b, kk, at, pts):
                mm512(at[0:65, :], v_sb[:, b, kk, h, 0:65], pts[kk][:],
                      start=(kk == live_kk[b][0]),
                      stop=(kk == live_kk[b][-1]))

            def normalize(qq, h, b, at):
                """Reciprocal of the denom row; multiply into packed au_sb."""
                dn = rrp.tile([1, 1024], f32, tag="dn", name="dn")
                nc.vector.tensor_copy(out=dn[:], in_=at[64:65, :])
                rr = rrp.tile([1, 1024], f32, tag="rr", name="rr")
                nc.vector.reciprocal_approx_fast(out=rr[:], in_=dn[:])
                bcs = bcp.tile([64, 1024], f32, tag="bcs", name="bcs")
                nc.gpsimd.partition_broadcast(bcs[:], rr[:])
                nc.vector.tensor_mul(
                    out=au_sb[h * 64 : (h + 1) * 64, b, qq, :],
                    in0=at[0:64, :],
                    in1=bcs[:],
                )

            # ---------------- the schedule ----------------
            def window(cur, arr, prev=None, prev_pts=None, fillers=(),
                       inline_at=None, inline_lag=3, ring3=None):
                """cur=(qq,h,b); prev=(qq,h,b, at) to run avs for; fillers
                pop one per two kk steps; inline_at: run cur's own avs lagged
                behind the score stream (final pass, shrinks the drain)."""
                qq, h, b = cur
                fillers = list(fillers)
                pts = {}
                live = live_kk[b]
                prev_live = live_kk[prev[2]] if prev else []
                for i, kk in enumerate(live):
                    # av of previous pass first: keeps PE fed if sc stalls
                    if prev and i < len(prev_live):
                        av_step(prev[0], prev[1], prev[2], prev_live[i],
                                prev[3], prev_pts)
                    score_step(qq, h, b, kk, arr, pts, i, ring3)
                    if inline_at is not None and i >= inline_lag:
                        av_step(qq, h, b, live[i - inline_lag], inline_at, pts)
                    if fillers and i % 2 == 1:
                        fillers.pop(0)()
                # leftover prev avs (when prev has more live kk than cur)
                for i in range(len(live), len(prev_live)):
                    av_step(prev[0], prev[1], prev[2], prev_live[i],
                            prev[3], prev_pts)
                while fillers:
                    fillers.pop(0)()
                if prev:
                    normalize(prev[0], prev[1], prev[2], prev[3])
                if inline_at is not None:
                    for i in range(max(0, len(live) - inline_lag), len(live)):
                        av_step(qq, h, b, live[i], inline_at, pts)
                return pts

            # ramp: weights + fused qk(b0) x-streams own the DMA device;
            # bias preps are gated and trickle via the Pool software DGE
            for nm in ("wq", "wk", "wv"):
                nc.sync.dma_start(out=w_sb[nm][:], in_=d[nm + "T"][:])
            nc.sync.dma_start(out=bq_sb[:], in_=d["bq"][:])
            nc.gpsimd.memset(warm[:], 0.0)
            nc.scalar.activation(out=warm[:], in_=warm[:], func=Exp)
            ptk0, ptk1 = ptile(psS, "pk00"), ptile(psS, "pk01")
            ptq0, ptq1 = ptile(psX, "pq00"), ptile(psT, "pq01")
            proj_kq0(ptk0, ptk1, ptq0, ptq1)
            nc.sync.dma_start(out=vm_sb[:], in_=d["vmask"][:])
            for h in range(HPC):
                nc.sync.dma_start(out=v_sb[:, :, :, h, 64:65], in_=d["vcol"][:])
            load_bias(0, 0, btA, nc.gpsimd, gate=qT_sb[0:1, 0, 0:NK])
            load_bias(0, 1, btB, nc.gpsimd)

            # w0: scores(0,0,0), 3-deep score ring via psT rotation
            ptsA = window((0, 0, 0), btA, ring3=psT)
            nc.sync.dma_start(out=wo_sb[:], in_=d["woT"][:])

            # v(b0) then k(b1) blocks (x chunks prefetched during w0)
            pvA, pvB = ptile(psT, "pv0A"), ptile(psX, "pv0B")
            proj_v(0, pvA, pvB)
            ktB, ktA = ptile(psT, "kt1B"), ptile(psX, "kt1A")
            proj_k1(ktA, ktB)
            atA = ptile(psT, "at000")

            # w1: scores(0,1,0) + avs(0,0,0)
            ptsB = window((0, 1, 0), btB, prev=(0, 0, 0, atA), prev_pts=ptsA)

            # q(b1) block
            qtA, qtB = ptile(psT, "qt1A"), ptile(psX, "qt1B")
            proj_q1(qtA, qtB)
            atB = ptile(psX, "at010")

            # w2: scores(0,0,1) + avs(0,1,0)
            ptsC = window((0, 0, 1), btA, prev=(0, 1, 0, atB), prev_pts=ptsB)

            # v(b1) block
            pv1B, pv1A = ptile(psT, "pv1B"), ptile(psX, "pv1A")
            proj_v(1, pv1A, pv1B)
            atC = ptile(psT, "at001")

            # w3: scores(0,1,1) + avs(0,0,1) + oproj(0,b0); refill A <- (1,0)
            load_bias(1, 0, btA, nc.gpsimd)
            ptsD = window((0, 1, 1), btB, prev=(0, 0, 1, atC), prev_pts=ptsC,
                          fillers=[oproj_chunk(0, 0, do, "vector", psX)
                                   for do in range(3)])
            atD = ptile(psX, "at011")

            # w4: scores(1,0,0) + avs(0,1,1) + oproj(0,b0); refill B <- (1,1)
            load_bias(1, 1, btB, nc.gpsimd)
            ptsE = window((1, 0, 0), btA, prev=(0, 1, 1, atD), prev_pts=ptsD,
                          fillers=[oproj_chunk(0, 0, do, "act", psT)
                                   for do in range(3, 6)])
            atE = ptile(psT, "at100")

            # w5: scores(1,1,0) + avs(1,0,0) + oproj
            ptsF = window((1, 1, 0), btB, prev=(1, 0, 0, atE), prev_pts=ptsE,
                          fillers=[oproj_chunk(0, 0, do, "act", psX)
                                   for do in range(6, 8)]
                          + [oproj_chunk(0, 1, do, "vector", psX)
                             for do in range(3)])
            atF = ptile(psX, "at110")

            # w6: scores(1,0,1) + avs(1,1,0) + oproj(0,b1)
            ptsG = window((1, 0, 1), btA, prev=(1, 1, 0, atF), prev_pts=ptsF,
                          fillers=[oproj_chunk(0, 1, do, "act" if do % 2 else
                                               "vector", psT)
                                   for do in range(3, 8)])
            atG = ptile(psT, "at101")

            # w7: scores(1,1,1) + avs(1,0,1) + inline avs(1,1,1) + oproj(1,b0)
            atH = ptile(psX, "at111")
            window((1, 1, 1), btB, prev=(1, 0, 1, atG), prev_pts=ptsG,
                   fillers=[oproj_chunk(1, 0, do, "act" if do % 2 else
                                        "vector")
                            for do in range(8)],
                   inline_at=atH)

            # tail: normalize + oproj(1,b1)
            normalize(1, 1, 1, atH)
            for do in range(ND):
                oproj_chunk(1, 1, do, "act" if do % 2 else "vector",
                            psT if do % 2 else psS)()
    if not nc.is_finalized():
        nc.finalize()
    return nc


def _mask_key(mask):
    """Classify (b, kk) tiles: 'full' = all masked out, 'part' = partially."""
    full, part = set(), set()
    for b in range(B):
        m = mask[b].reshape(NK, 128)
        for kk in range(NK):
            n = int(m[kk].sum())
            if n == 0:
                full.add((b, kk))
            elif n < 128:
                part.add((b, kk))
    return frozenset(full), frozenset(part)


def kernel(query, key, value, key_padding_mask, relative_bias,
           Wq, bq, Wk, bk, Wv, bv, Wo, bo, **_unused):
    query = np.asarray(query, dtype=np.float32)
    key = np.asarray(key, dtype=np.float32)
    value = np.asarray(value, dtype=np.float32)
    mask = np.asarray(key_padding_mask)
    relative_bias = np.asarray(relative_bias, dtype=np.float32)
    Wq, bq = np.asarray(Wq, np.float32), np.asarray(bq, np.float32)
    Wk = np.asarray(Wk, np.float32)
    Wv, bv = np.asarray(Wv, np.float32), np.asarray(bv, np.float32)
    Wo, bo = np.asarray(Wo, np.float32), np.asarray(bo, np.float32)

    queryT = np.ascontiguousarray(query.transpose(0, 2, 1)).astype(ml_dtypes.bfloat16)
    keyT = np.ascontiguousarray(key.transpose(0, 2, 1)).astype(ml_dtypes.bfloat16)
    valueT = np.ascontiguousarray(value.transpose(0, 2, 1)).astype(ml_dtypes.bfloat16)
    maskf = mask.astype(np.float32)  # (B, S) 1.0 live / 0.0 masked
    vmask = np.ascontiguousarray(
        maskf.reshape(B, NK, 128).transpose(2, 0, 1)
    )  # (128, B, NK)
    vcol = vmask[:, :, :, None].astype(ml_dtypes.bfloat16)  # (128, B, NK, 1)
    ebiasT = np.exp(
        relative_bias[0].transpose(0, 2, 1)
    ).astype(ml_dtypes.bfloat16)  # (H, S_k, S_q)
    sc = 1.0 / np.sqrt(DH)
    # bv's effect: softmax rows sum to 1 -> out += Wo @ bv (host); bk cancels.
    bo_eff = bo + Wo @ bv

    in_maps = []
    for c in range(NC):
        hs = slice(c * HPC * DH, (c + 1) * HPC * DH)  # this core's 128 head rows
        in_maps.append({
            "queryT": queryT, "keyT": keyT, "valueT": valueT,
            "ebiasT": np.ascontiguousarray(ebiasT[c * HPC : (c + 1) * HPC]),
            "vcol": vcol, "vmask": vmask,
            "wqT": np.ascontiguousarray(((Wq[hs] * sc).T).reshape(8, 128, 128).transpose(1, 0, 2).reshape(128, -1)).astype(ml_dtypes.bfloat16),
            "wkT": np.ascontiguousarray((Wk[hs].T).reshape(8, 128, 128).transpose(1, 0, 2).reshape(128, -1)).astype(ml_dtypes.bfloat16),
            "wvT": np.ascontiguousarray((Wv[hs].T).reshape(8, 128, 128).transpose(1, 0, 2).reshape(128, -1)).astype(ml_dtypes.bfloat16),
            "bq": (bq[hs] * sc).reshape(128, 1).astype(np.float32),
            # packed: row r<64 = h0 dim r, r>=64 = h1 dim r-64; cols = D out
            "woT": np.ascontiguousarray(Wo[:, hs].T).astype(ml_dtypes.bfloat16),
        })

    global _LAST_IN_MAPS, _LAST_KEY
    _LAST_IN_MAPS = in_maps
    keyk = _mask_key(mask)
    _LAST_KEY = keyk
    if keyk not in _PROGRAMS:
        _PROGRAMS[keyk] = _build_program(*keyk)
    res = run_bass_kernel_spmd(_PROGRAMS[keyk], in_maps, list(range(NC)))
    acc = np.zeros((B, D, S), dtype=np.float32)
    for r in res.results:
        acc += r["oT"].astype(np.float32)
    return acc.transpose(0, 2, 1) + bo_eff


def run_profiled(inputs=None):
    """Timeline-simulator timing (cost-model) for the cached program, ns."""
    from concourse.timeline_sim import TimelineSim

    nc = _PROGRAMS[_LAST_KEY]
    sim = TimelineSim(nc, trace=False)
    return int(sim.simulate())


# revision 32
# speedup vs baseline: 1.0189x; 1.0189x over previous
"""MultiHeadAttention with relative bias + key padding mask on 8 trn2 NeuronCores.

Sharding: head-parallel — core c owns head pair {2c, 2c+1} for BOTH batches.
Each core computes its heads' attention and a partial o-projection over the
full output dim; the host sums the 8 partials and adds bo_eff.

Device-side formulation (per core, per batch b, per head h):
  qT = (Wq_h/8) @ query_b^T + bq/8     [64, S]  (1/sqrt(DH) folded into Wq,bq)
  kT =  Wk_h    @ key_b^T              [64, S]  (bk dropped: cancels in softmax)
  v  = value_b @ Wv_h^T  directly in [s, dh] layout (lhsT = x^T tiles), with
       masked key rows zeroed and a mask-column appended (denominator trick);
       bv dropped: softmax rows sum to 1, so its effect is bo += Wo @ bv (host).
  scoresT[kk,qq] = kT^T-slice . qT-slice                  (PE, f32r)
  PT = exp(scoresT) ⊙ exp(biasT)       (ACT exp -> bf16, DVE 2x bf16 multiply;
                                        exp(bias) precomputed on the host)
  attnT[dh,qq] (+ denom row via mask column in v) = v_aug^T @ PT
  attnT *= broadcast(1/denom)          (DVE fast-approx recip on the SBUF-staged
                                        denom row + gpsimd partition-broadcast)
  oT_partial[dout,s] += WoT_h . attnT  (K=64 per head)

Schedule: batch-0 attention passes start as soon as k/q(b0) project, covering
the batch-1 input DMA window; (1,0) batch-0 scores fill the kq(b1) transfer
window; o-projections interleave into later passes' av loops. Bias tiles for
the (0,*) blocks are resident in SBUF so the batch-1 passes reuse them; the
v-projection runs in the middle of the first score stream (av matmuls are
deferred behind buffered P tiles to keep the in-order PE queue from stalling).

Fully-masked (b, kk) tiles are skipped at program-build time (the program is
cached keyed on the observed mask tile pattern). relative_bias is exp()'d,
pre-transposed and cast to bf16 on the host.
"""
import sys

sys.path.insert(0, "/opt/trn_rl_repo")
import numpy as np
import ml_dtypes

import concourse.bass as bass
from concourse import bacc
import concourse.tile as tile
from concourse import mybir
from concourse.bass_utils import run_bass_kernel_spmd

B, S, D, H, DH = 2, 2048, 1024, 16, 64
NC = 8
HPC = H // NC  # heads per core = 2
f32 = mybir.dt.float32
bf16 = mybir.dt.bfloat16
f32r = mybir.dt.float32r
Exp = mybir.ActivationFunctionType.Exp
Ln = mybir.ActivationFunctionType.Ln
NK = S // 128  # 16 k-tiles of 128
ND = D // 128  # 8 chunks of the model dim

_PROGRAMS = {}  # keyed by mask tile pattern
_LAST_IN_MAPS = None
_LAST_KEY = None


def _build_program(full_tiles, part_tiles):
    """full_tiles: frozenset of fully-masked (b, kk); part_tiles: frozenset of
    partially-masked (b, kk) needing per-tile v-row zeroing."""
    nc = bacc.Bacc(None, target_bir_lowering=False)
    d = {}
    d["queryT"] = nc.declare_dram_parameter("queryT", [B, D, S], bf16, isOutput=False)
    d["keyT"] = nc.declare_dram_parameter("keyT", [B, D, S], bf16, isOutput=False)
    d["valueT"] = nc.declare_dram_parameter("valueT", [B, D, S], bf16, isOutput=False)
    d["ebiasT"] = nc.declare_dram_parameter("ebiasT", [HPC, S, S], bf16, isOutput=False)
    d["vcol"] = nc.declare_dram_parameter("vcol", [128, B, NK, 1], bf16, isOutput=False)
    d["vmask"] = nc.declare_dram_parameter("vmask", [128, B, NK], f32, isOutput=False)
    d["wqT"] = nc.declare_dram_parameter("wqT", [128, ND * 128], bf16, isOutput=False)
    d["wkT"] = nc.declare_dram_parameter("wkT", [128, ND * 128], bf16, isOutput=False)
    d["wvT"] = nc.declare_dram_parameter("wvT", [128, ND * 128], bf16, isOutput=False)
    d["bq"] = nc.declare_dram_parameter("bq", [128, 1], f32, isOutput=False)
    d["woT"] = nc.declare_dram_parameter("woT", [128, D], bf16, isOutput=False)
    oT = nc.declare_dram_parameter("oT", [B, D, S], bf16, isOutput=True)

    # per-batch live kk lists (at least one live kk per batch is assumed)
    live_kk = {b: [kk for kk in range(NK) if (b, kk) not in full_tiles]
               for b in range(B)}

    with tile.TileContext(nc) as tc:
        with (
            tc.tile_pool(name="const", bufs=1) as const,
            tc.tile_pool(name="persist", bufs=1) as persist,
            tc.tile_pool(name="xt", bufs=4) as xt,
            tc.tile_pool(name="btp", bufs=3) as btp,
            tc.tile_pool(name="etp", bufs=8) as etp,
            tc.tile_pool(name="ptw", bufs=16) as ptw,
            tc.tile_pool(name="otp", bufs=4) as otp,
            tc.tile_pool(name="rrp", bufs=2) as rrp,
            tc.tile_pool(name="bcp", bufs=2) as bcp,
            tc.tile_pool(name="psS", bufs=2, space="PSUM") as psS,
            tc.tile_pool(name="psT", bufs=2, space="PSUM") as psT,
        ):
            w_sb = {}
            for nm in ("wq", "wk", "wv"):
                w_sb[nm] = const.tile([128, ND, 128], bf16, tag=nm, name="w_" + nm)
                nc.sync.dma_start(out=w_sb[nm][:], in_=d[nm + "T"][:])
            bq_sb = const.tile([128, 1], f32, tag="bq", name="bq_sb")
            nc.sync.dma_start(out=bq_sb[:], in_=d["bq"][:])
            wo_sb = const.tile([128, D], bf16, tag="wo", name="wo_sb")
            vm_sb = const.tile([128, B, NK], f32, tag="vm", name="vm_sb")

            qT_sb = persist.tile([128, B, S], bf16, tag="qT", name="qT_sb")
            kT_sb = persist.tile([128, B, S], bf16, tag="kT", name="kT_sb")
            v_sb = persist.tile([128, B, NK, HPC, 66], bf16, tag="v", name="v_sb")
            au_sb = persist.tile([128, B, S], bf16, tag="au", name="au_sb")

            # Resident bias arrays for batch-0-early blocks (32 KB/part each)
            btresA = persist.tile([128, NK, 1024], bf16, tag="btA", name="btresA")
            btresB = persist.tile([128, NK, 1024], bf16, tag="btB", name="btresB")

            # ---------------- Phase 1: projections (bf16 inputs) ----------------
            # Order k, v, q so attention-critical tensors land first.
            def proj_kq(b):
                # --- k projection -> kT_sb (no bias: cancels in softmax) ---
                ptk = [psS.tile([128, 1024], f32, tag="mm", name=f"pk{b}{i}")
                       for i in range(2)]
                for dc in range(ND):
                    xc = xt.tile([128, S], bf16, tag="xc", name="xck")
                    nc.sync.dma_start(
                        out=xc[:], in_=d["keyT"][b, dc * 128 : (dc + 1) * 128, :]
                    )
                    lastc = (live_kk[b][-1] + 1) * 128  # cols beyond: never read
                    for qh in range(2):
                        for hf in range(2):
                            c0 = qh * 1024 + hf * 512
                            c1 = min(c0 + 512, lastc)
                            if c1 <= c0:
                                continue
                            nc.tensor.matmul(
                                out=ptk[qh][:, hf * 512 : hf * 512 + (c1 - c0)],
                                lhsT=w_sb["wk"][:, dc, :],
                                rhs=xc[:, c0 : c1],
                                start=(dc == 0),
                                stop=(dc == ND - 1),
                            )
                nc.scalar.copy(out=kT_sb[:, b, 0:1024], in_=ptk[0][:])
                nc.vector.tensor_copy(out=kT_sb[:, b, 1024:2048], in_=ptk[1][:])
                # --- q projection -> qT_sb (+ bq) ---
                ptq = [psS.tile([128, 1024], f32, tag="mm", name=f"pq{b}{i}")
                       for i in range(2)]
                for dc in range(ND):
                    xc = xt.tile([128, S], bf16, tag="xc", name="xcq")
                    nc.sync.dma_start(
                        out=xc[:], in_=d["queryT"][b, dc * 128 : (dc + 1) * 128, :]
                    )
                    for qh in range(2):
                        for hf in range(2):
                            nc.tensor.matmul(
                                out=ptq[qh][:, hf * 512 : (hf + 1) * 512],
                                lhsT=w_sb["wq"][:, dc, :],
                                rhs=xc[:, qh * 1024 + hf * 512 : qh * 1024 + (hf + 1) * 512],
                                start=(dc == 0),
                                stop=(dc == ND - 1),
                            )
                nc.scalar.add(out=qT_sb[:, b, 0:1024], in_=ptq[0][:], add=bq_sb[:])
                nc.vector.tensor_scalar_add(
                    out=qT_sb[:, b, 1024:2048], in0=ptq[1][:], scalar1=bq_sb[:]
                )

            def proj_v(b):
                # --- v projection, direct [s, dh] layout ---
                psv = [psT.tile([128, 1024], f32, tag="at", name=f"pv{b}{i}")
                       for i in range(2)]
                for dc in range(ND):
                    xc = xt.tile([128, S], bf16, tag="xc", name="xcv")
                    nc.sync.dma_start(
                        out=xc[:], in_=d["valueT"][b, dc * 128 : (dc + 1) * 128, :]
                    )
                    for st in range(NK):
                        # start_tensor_calc zeroes the whole 2KB PSUM bank (4
                        # st-regions): only the bank-first st may set it. Skip
                        # fully-masked st tiles unless needed for bank zeroing.
                        if (b, st) in full_tiles and (
                            st % 4 != 0
                            or all((b, s) in full_tiles
                                   for s in range(st, min(st + 4, NK)))
                        ):
                            continue
                        nc.tensor.matmul(
                            out=psv[st // 8][:, (st % 8) * 128 : (st % 8 + 1) * 128],
                            lhsT=xc[:, st * 128 : (st + 1) * 128],
                            rhs=w_sb["wv"][:, dc, :],
                            start=(dc == 0 and st % 4 == 0),
                            stop=(dc == ND - 1),
                            skip_group_check=True,
                        )
                # copy into v_sb (bf16), zeroing masked key rows where needed
                for half in range(2):
                    sts = [st for st in range(half * 8, (half + 1) * 8)]
                    simple = [st for st in sts
                              if (b, st) not in part_tiles and (b, st) not in full_tiles]
                    # bulk-copy the longest contiguous prefix run of simple tiles
                    run = []
                    for st in sts:
                        if st in simple and (not run or st == run[-1] + 1):
                            run.append(st)
                        elif not run:
                            continue
                        else:
                            break
                    if run:
                        st0, n = run[0], len(run)
                        nc.vector.tensor_copy(
                            out=v_sb[:, b, st0 : st0 + n, :, 0:64],
                            in_=psv[half][
                                :, (st0 - half * 8) * 128 : (st0 - half * 8 + n) * 128
                            ].rearrange("p (t h m) -> p t h m", t=n, h=HPC),
                        )
                    for st in sts:
                        if st in run or (b, st) in full_tiles:
                            continue
                        i0 = (st - half * 8) * 128
                        if (b, st) in part_tiles:
                            nc.vector.tensor_scalar_mul(
                                out=v_sb[:, b, st, :, 0:64],
                                in0=psv[half][:, i0 : i0 + 128].rearrange(
                                    "p (h m) -> p h m", h=HPC
                                ),
                                scalar1=vm_sb[:, b, st : st + 1],
                            )
                        else:
                            nc.vector.tensor_copy(
                                out=v_sb[:, b, st, :, 0:64],
                                in_=psv[half][:, i0 : i0 + 128].rearrange(
                                    "p (h m) -> p h m", h=HPC
                                ),
                            )
            # ------- Phase 2: attention + fused norm; o-proj interleaved -------
            def oproj_chunk(qq, b, do, pool=None):
                q0 = qq * 1024

                def emit():
                    po = (pool or psS).tile([128, 1024], f32,
                                            tag="mm" if (pool or psS) is psS else "at",
                                            name="po")
                    for hf in range(2):
                        hs512 = slice(hf * 512, (hf + 1) * 512)
                        nc.tensor.matmul(
                            out=po[:, hs512],
                            lhsT=wo_sb[:, do * 128 : (do + 1) * 128],
                            rhs=au_sb[
                                :, b, q0 + hf * 512 : q0 + (hf + 1) * 512
                            ],
                            start=True,
                            stop=True,
                        )
                    ot = otp.tile([128, 1024], bf16, tag="ot", name="ot")
                    if do % 2:
                        nc.scalar.copy(out=ot[:], in_=po[:])
                    else:
                        nc.vector.tensor_copy(out=ot[:], in_=po[:])
                    nc.sync.dma_start(
                        out=oT[b, do * 128 : (do + 1) * 128, q0 : q0 + 1024],
                        in_=ot[:],
                    )
                return emit

            def emit_block(qq, h, ochunks):
                """kk loop for (qq, h); pops one deferred o-proj chunk per kk."""
                q0 = qq * 1024
                at = [psT.tile([128, 1024], f32, tag="at", name=f"at{_i}")
                      for _i in range(B)]
                for kk in range(NK):
                    live = [b for b in range(B) if (b, kk) not in full_tiles]
                    if not live:
                        if ochunks:
                            ochunks.pop(0)()
                        continue
                    bt = btp.tile([128, 1024], bf16, tag="bt", name="bt")
                    nc.sync.dma_start(
                        out=bt[:],
                        in_=d["ebiasT"][h, kk * 128 : (kk + 1) * 128, q0 : q0 + 1024],
                    )
                    for b in live:
                        sc = psS.tile([128, 1024], f32, tag="mm", name="sc")
                        for hf in range(2):
                            hs512 = slice(hf * 512, (hf + 1) * 512)
                            nc.tensor.matmul(
                                out=sc[:, hs512],
                                lhsT=kT_sb[
                                    h * 64 : (h + 1) * 64, b, kk * 128 : (kk + 1) * 128
                                ],
                                rhs=qT_sb[
                                    h * 64 : (h + 1) * 64, b,
                                    q0 + hf * 512 : q0 + (hf + 1) * 512
                                ],
                                start=True, stop=True,
                            )
                        et = etp.tile([128, 1024], bf16, tag="et", name="et")
                        nc.scalar.activation(out=et[:], in_=sc[:], func=Exp)
                        pt = ptw.tile([128, 1024], bf16, tag="ptw", name="pt")
                        nc.vector.tensor_mul(out=pt[:], in0=et[:], in1=bt[:])
                        for hf in range(2):
                            hs512 = slice(hf * 512, (hf + 1) * 512)
                            nc.tensor.matmul(
                                out=at[b][0:65, hs512],
                                lhsT=v_sb[:, b, kk, h, 0:65],
                                rhs=pt[:, hs512],
                                start=(kk == live_kk[b][0]),
                                stop=(kk == live_kk[b][-1]),
                            )
                    if ochunks:
                        ochunks.pop(0)()
                # normalize: recip of denom row, broadcast, multiply -> au_sb
                for b in range(B):
                    rr = rrp.tile([1, 1024], f32, tag="rr", name="rr")
                    nc.scalar.activation(out=rr[:], in_=at[b][64:65, :], func=Ln)
                    nc.scalar.activation(out=rr[:], in_=rr[:], func=Exp, scale=-1.0)
                    bcs = bcp.tile([64, 1024], f32, tag="bcs", name="bcs")
                    nc.gpsimd.partition_broadcast(bcs[:], rr[:])
                    nc.vector.tensor_mul(
                        out=au_sb[h * 64 : (h + 1) * 64, b, q0 : q0 + 1024],
                        in0=at[b][0:64, :],
                        in1=bcs[:],
                    )
                while ochunks:
                    ochunks.pop(0)()

            def load_resident_bias(qq, h, btres, eng):
                """Dispatch all bias-tile DMAs for (qq, h) into btres upfront
                on the given engine queue (ACT during idle front, SP later)."""
                q0 = qq * 1024
                for kk in range(NK):
                    if any((bb, kk) not in full_tiles for bb in range(B)):
                        eng.dma_start(
                            out=btres[:, kk, :],
                            in_=d["ebiasT"][h, kk * 128 : (kk + 1) * 128,
                                            q0 : q0 + 1024],
                        )

            def attn_pass(qq, h, b, btres, ochunks=(), mid_emit=None,
                          defer=False):
                """Single-batch kk pass for (qq, h); bias read from the
                resident array btres. With bias_jit, each bias tile's DMA is
                dispatched from the ACT hwdge queue a few iterations ahead.
                The P tiles for all kk are buffered so the av matmuls trail
                the score/exp stream (avoids in-order PE stalls on v
                availability). Normalizes at the end."""
                q0 = qq * 1024
                at = psT.tile([128, 1024], f32, tag="at", name=f"at{qq}{h}{b}")
                ochunks = list(ochunks)
                bts = {}
                pts = {}
                for i, kk in enumerate(live_kk[b]):
                    if btres is None:
                        bt = btp.tile([128, 1024], bf16, tag="bt", name="bt")
                        nc.sync.dma_start(
                            out=bt[:],
                            in_=d["ebiasT"][h, kk * 128 : (kk + 1) * 128,
                                            q0 : q0 + 1024],
                        )
                        bts[kk] = bt
                    sc = psS.tile([128, 1024], f32, tag="mm", name="sc")
                    for hf in range(2):
                        hs512 = slice(hf * 512, (hf + 1) * 512)
                        nc.tensor.matmul(
                            out=sc[:, hs512],
                            lhsT=kT_sb[
                                h * 64 : (h + 1) * 64, b, kk * 128 : (kk + 1) * 128
                            ],
                            rhs=qT_sb[
                                h * 64 : (h + 1) * 64, b,
                                q0 + hf * 512 : q0 + (hf + 1) * 512
                            ],
                            start=True, stop=True,
                        )
                    et = etp.tile([128, 1024], bf16, tag="et", name="et")
                    nc.scalar.activation(out=et[:], in_=sc[:], func=Exp)
                    pt = ptw.tile([128, 1024], bf16, tag="ptw", name="ptw")
                    src_bt = bts[kk][:] if btres is None else btres[:, kk, :]
                    nc.vector.tensor_mul(out=pt[:], in0=et[:], in1=src_bt)
                    pts[kk] = pt
                def finish():
                    if mid_emit is not None:
                        mid_emit()
                    for kk in live_kk[b]:
                        for hf in range(2):
                            hs512 = slice(hf * 512, (hf + 1) * 512)
                            nc.tensor.matmul(
                                out=at[0:65, hs512],
                                lhsT=v_sb[:, b, kk, h, 0:65],
                                rhs=pts[kk][:, hs512],
                                start=(kk == live_kk[b][0]),
                                stop=(kk == live_kk[b][-1]),
                            )
                        if ochunks:
                            ochunks.pop(0)()
                    # normalize: recip of denom row, broadcast, multiply
                    dn = rrp.tile([1, 1024], f32, tag="dn", name="dn")
                    nc.vector.tensor_copy(out=dn[:], in_=at[64:65, :])
                    rr = rrp.tile([1, 1024], f32, tag="rr", name="rr")
                    nc.vector.reciprocal_approx_fast(out=rr[:], in_=dn[:])
                    bcs = bcp.tile([64, 1024], f32, tag="bcs", name="bcs")
                    nc.gpsimd.partition_broadcast(bcs[:], rr[:])
                    nc.vector.tensor_mul(
                        out=au_sb[h * 64 : (h + 1) * 64, b, q0 : q0 + 1024],
                        in0=at[0:64, :],
                        in1=bcs[:],
                    )
                    for f in ochunks:
                        f()
                if defer:
                    return finish
                finish()

            proj_kq(0)
            nc.sync.dma_start(out=vm_sb[:], in_=d["vmask"][:])
            for h in range(HPC):
                nc.sync.dma_start(out=v_sb[:, :, :, h, 64:65], in_=d["vcol"][:])
            load_resident_bias(0, 0, btresA, nc.sync)
            # scores(0,0,0) stream first; v(0) projection + avs trail it
            attn_pass(0, 0, 0, btresA, mid_emit=lambda: proj_v(0))
            load_resident_bias(0, 1, btresB, nc.sync)
            attn_pass(0, 1, 0, btresB)
            proj_kq(1)
            # batch-0 pass of (1,0) fills the b1-projection DMA window:
            # its exps have no new deps; bias arrives later for the mults.
            # Its avs are deferred past C's score stream so the late bias
            # doesn't block the in-order PE queue.
            g_fin = attn_pass(1, 0, 0, None, defer=True)
            nc.sync.dma_start(out=wo_sb[:], in_=d["woT"][:])
            c_fin = attn_pass(0, 0, 1, btresA, defer=True)
            g_fin()
            proj_v(1)
            c_fin()
            attn_pass(0, 1, 1, btresB)
            load_resident_bias(1, 0, btresB, nc.sync)
            attn_pass(1, 0, 1, btresB,
                      ochunks=[oproj_chunk(0, b, do)
                               for do in range(ND) for b in range(B)])
            load_resident_bias(1, 1, btresA, nc.sync)
            attn_pass(1, 1, 0, btresA)
            attn_pass(1, 1, 1, btresA,
                      ochunks=[oproj_chunk(1, 0, do) for do in range(ND)])
            for do in range(ND):
                oproj_chunk(1, 1, do, pool=psT if do % 2 else psS)()
    if not nc.is_finalized():
        nc.finalize()
    return nc


def _mask_key(mask):
    """Classify (b, kk) tiles: 'full' = all masked out, 'part' = partially."""
    full, part = set(), set()
    for b in range(B):
        m = mask[b].reshape(NK, 128)
        for kk in range(NK):
            n = int(m[kk].sum())
            if n == 0:
                full.add((b, kk))
            elif n < 128:
                part.add((b, kk))
    return frozenset(full), frozenset(part)


def kernel(query, key, value, key_padding_mask, relative_bias,
           Wq, bq, Wk, bk, Wv, bv, Wo, bo, **_unused):
    query = np.asarray(query, dtype=np.float32)
    key = np.asarray(key, dtype=np.float32)
    value = np.asarray(value, dtype=np.float32)
    mask = np.asarray(key_padding_mask)
    relative_bias = np.asarray(relative_bias, dtype=np.float32)
    Wq, bq = np.asarray(Wq, np.float32), np.asarray(bq, np.float32)
    Wk = np.asarray(Wk, np.float32)
    Wv, bv = np.asarray(Wv, np.float32), np.asarray(bv, np.float32)
    Wo, bo = np.asarray(Wo, np.float32), np.asarray(bo, np.float32)

    queryT = np.ascontiguousarray(query.transpose(0, 2, 1)).astype(ml_dtypes.bfloat16)
    keyT = np.ascontiguousarray(key.transpose(0, 2, 1)).astype(ml_dtypes.bfloat16)
    valueT = np.ascontiguousarray(value.transpose(0, 2, 1)).astype(ml_dtypes.bfloat16)
    maskf = mask.astype(np.float32)  # (B, S) 1.0 live / 0.0 masked
    vmask = np.ascontiguousarray(
        maskf.reshape(B, NK, 128).transpose(2, 0, 1)
    )  # (128, B, NK)
    vcol = vmask[:, :, :, None].astype(ml_dtypes.bfloat16)  # (128, B, NK, 1)
    ebiasT = np.exp(
        relative_bias[0].transpose(0, 2, 1)
    ).astype(ml_dtypes.bfloat16)  # (H, S, S) keys-major
    sc = 1.0 / np.sqrt(DH)
    # bv's effect: softmax rows sum to 1 -> out += Wo @ bv (host); bk cancels.
    bo_eff = bo + Wo @ bv

    in_maps = []
    for c in range(NC):
        hs = slice(c * HPC * DH, (c + 1) * HPC * DH)  # this core's 128 head rows
        in_maps.append({
            "queryT": queryT, "keyT": keyT, "valueT": valueT,
            "ebiasT": np.ascontiguousarray(ebiasT[c * HPC : (c + 1) * HPC]),
            "vcol": vcol, "vmask": vmask,
            "wqT": np.ascontiguousarray(((Wq[hs] * sc).T).reshape(8, 128, 128).transpose(1, 0, 2).reshape(128, -1)).astype(ml_dtypes.bfloat16),
            "wkT": np.ascontiguousarray((Wk[hs].T).reshape(8, 128, 128).transpose(1, 0, 2).reshape(128, -1)).astype(ml_dtypes.bfloat16),
            "wvT": np.ascontiguousarray((Wv[hs].T).reshape(8, 128, 128).transpose(1, 0, 2).reshape(128, -1)).astype(ml_dtypes.bfloat16),
            "bq": (bq[hs] * sc).reshape(128, 1).astype(np.float32),
            "woT": np.ascontiguousarray(Wo[:, hs].T).astype(ml_dtypes.bfloat16),
        })

    global _LAST_IN_MAPS, _LAST_KEY
    _LAST_IN_MAPS = in_maps
    keyk = _mask_key(mask)
    _LAST_KEY = keyk
    if keyk not in _PROGRAMS:
        _PROGRAMS[keyk] = _build_program(*keyk)
    res = run_bass_kernel_spmd(_PROGRAMS[keyk], in_maps, list(range(NC)))
    acc = np.zeros((B, D, S), dtype=np.float32)
    for r in res.results:
        acc += r["oT"].astype(np.float32)
    return acc.transpose(0, 2, 1) + bo_eff


def run_profiled(inputs=None):
    """Timeline-simulator timing (cost-model) for the cached program, ns."""
    from concourse.timeline_sim import TimelineSim

    nc = _PROGRAMS[_LAST_KEY]
    sim = TimelineSim(nc, trace=False)
    return int(sim.simulate())



# revision 33
# speedup vs baseline: 1.0311x; 1.0120x over previous
"""MultiHeadAttention with relative bias + key padding mask on 8 trn2 NeuronCores.

Sharding: head-parallel — core c owns head pair {2c, 2c+1} for BOTH batches.
Each core computes its heads' attention and a partial o-projection over the
full output dim; the host sums the 8 partials and adds bo_eff.

Device-side formulation (per core, per batch b, per head h):
  qT = (Wq_h/8) @ query_b^T + bq/8     [64, S]  (1/sqrt(DH) folded into Wq,bq)
  kT =  Wk_h    @ key_b^T              [64, S]  (bk dropped: cancels in softmax)
  v  = value_b @ Wv_h^T  directly in [s, dh] layout (lhsT = x^T tiles), with
       masked key rows zeroed and a mask-column appended (denominator trick);
       bv dropped: softmax rows sum to 1, so its effect is bo += Wo @ bv (host).
  scoresT[kk,qq] = kT^T-slice . qT-slice                  (PE, f32r)
  PT = exp(scoresT) ⊙ exp(biasT)       (ACT exp -> bf16, DVE 2x bf16 multiply;
                                        exp(bias) precomputed on the host)
  attnT[dh,qq] (+ denom row via mask column in v) = v_aug^T @ PT
  attnT *= broadcast(1/denom)          (DVE fast-approx recip on the SBUF-staged
                                        denom row + gpsimd partition-broadcast);
       the normalized rows land PACKED in au (h0 -> partitions 0:64, h1 ->
       64:128, via a DVE partition-offset write)
  oT_partial[dout,s] += woT_packed . au   (one K=128 accumulation per chunk
                                           instead of two K=64 ones)

Schedule: batch-0 attention passes start as soon as k/q(b0) project, covering
the batch-1 input DMA window; (1,0) batch-0 scores fill the kq(b1) transfer
window; o-projections interleave into later passes' av loops. Bias tiles for
the (0,*) blocks are resident in SBUF so the batch-1 passes reuse them; the
v-projection runs in the middle of the first score stream (av matmuls are
deferred behind buffered P tiles to keep the in-order PE queue from stalling).

Fully-masked (b, kk) tiles are skipped at program-build time (the program is
cached keyed on the observed mask tile pattern). relative_bias is exp()'d,
pre-transposed and cast to bf16 on the host.
"""
import sys

sys.path.insert(0, "/opt/trn_rl_repo")
import numpy as np
import ml_dtypes

import concourse.bass as bass
from concourse import bacc
import concourse.tile as tile
from concourse import mybir
from concourse.bass_utils import run_bass_kernel_spmd

B, S, D, H, DH = 2, 2048, 1024, 16, 64
NC = 8
HPC = H // NC  # heads per core = 2
f32 = mybir.dt.float32
bf16 = mybir.dt.bfloat16
f32r = mybir.dt.float32r
Exp = mybir.ActivationFunctionType.Exp
Ln = mybir.ActivationFunctionType.Ln
NK = S // 128  # 16 k-tiles of 128
ND = D // 128  # 8 chunks of the model dim

_PROGRAMS = {}  # keyed by mask tile pattern
_LAST_IN_MAPS = None
_LAST_KEY = None


def _build_program(full_tiles, part_tiles):
    """full_tiles: frozenset of fully-masked (b, kk); part_tiles: frozenset of
    partially-masked (b, kk) needing per-tile v-row zeroing."""
    nc = bacc.Bacc(None, target_bir_lowering=False)
    d = {}
    d["queryT"] = nc.declare_dram_parameter("queryT", [B, D, S], bf16, isOutput=False)
    d["keyT"] = nc.declare_dram_parameter("keyT", [B, D, S], bf16, isOutput=False)
    d["valueT"] = nc.declare_dram_parameter("valueT", [B, D, S], bf16, isOutput=False)
    d["ebiasT"] = nc.declare_dram_parameter("ebiasT", [HPC, S, S], bf16, isOutput=False)
    d["vcol"] = nc.declare_dram_parameter("vcol", [128, B, NK, 1], bf16, isOutput=False)
    d["vmask"] = nc.declare_dram_parameter("vmask", [128, B, NK], f32, isOutput=False)
    d["wqT"] = nc.declare_dram_parameter("wqT", [128, ND * 128], bf16, isOutput=False)
    d["wkT"] = nc.declare_dram_parameter("wkT", [128, ND * 128], bf16, isOutput=False)
    d["wvT"] = nc.declare_dram_parameter("wvT", [128, ND * 128], bf16, isOutput=False)
    d["bq"] = nc.declare_dram_parameter("bq", [128, 1], f32, isOutput=False)
    d["woT"] = nc.declare_dram_parameter("woT", [128, D], bf16, isOutput=False)
    oT = nc.declare_dram_parameter("oT", [B, D, S], bf16, isOutput=True)

    # per-batch live kk lists (at least one live kk per batch is assumed)
    live_kk = {b: [kk for kk in range(NK) if (b, kk) not in full_tiles]
               for b in range(B)}

    with tile.TileContext(nc) as tc:
        with (
            tc.tile_pool(name="const", bufs=1) as const,
            tc.tile_pool(name="persist", bufs=1) as persist,
            tc.tile_pool(name="xt", bufs=4) as xt,
            tc.tile_pool(name="btp", bufs=3) as btp,
            tc.tile_pool(name="etp", bufs=8) as etp,
            tc.tile_pool(name="ptw", bufs=16) as ptw,
            tc.tile_pool(name="otp", bufs=4) as otp,
            tc.tile_pool(name="rrp", bufs=2) as rrp,
            tc.tile_pool(name="bcp", bufs=2) as bcp,
            tc.tile_pool(name="psS", bufs=2, space="PSUM") as psS,
            tc.tile_pool(name="psT", bufs=2, space="PSUM") as psT,
        ):
            w_sb = {}
            for nm in ("wq", "wk", "wv"):
                w_sb[nm] = const.tile([128, ND, 128], bf16, tag=nm, name="w_" + nm)
                nc.sync.dma_start(out=w_sb[nm][:], in_=d[nm + "T"][:])
            bq_sb = const.tile([128, 1], f32, tag="bq", name="bq_sb")
            nc.sync.dma_start(out=bq_sb[:], in_=d["bq"][:])
            wo_sb = const.tile([128, D], bf16, tag="wo", name="wo_sb")
            vm_sb = const.tile([128, B, NK], f32, tag="vm", name="vm_sb")

            qT_sb = persist.tile([128, B, S], bf16, tag="qT", name="qT_sb")
            kT_sb = persist.tile([128, B, S], bf16, tag="kT", name="kT_sb")
            v_sb = persist.tile([128, B, NK, HPC, 66], bf16, tag="v", name="v_sb")
            au_sb = persist.tile([128, B, S], bf16, tag="au", name="au_sb")

            # Resident bias arrays for batch-0-early blocks (32 KB/part each)
            btresA = persist.tile([128, NK, 1024], bf16, tag="btA", name="btresA")
            btresB = persist.tile([128, NK, 1024], bf16, tag="btB", name="btresB")

            # ---------------- Phase 1: projections (bf16 inputs) ----------------
            # Order k, v, q so attention-critical tensors land first.
            def proj_kq(b):
                # --- k projection -> kT_sb (no bias: cancels in softmax) ---
                ptk = [psS.tile([128, 1024], f32, tag="mm", name=f"pk{b}{i}")
                       for i in range(2)]
                for dc in range(ND):
                    xc = xt.tile([128, S], bf16, tag="xc", name="xck")
                    nc.sync.dma_start(
                        out=xc[:], in_=d["keyT"][b, dc * 128 : (dc + 1) * 128, :]
                    )
                    lastc = (live_kk[b][-1] + 1) * 128  # cols beyond: never read
                    for qh in range(2):
                        for hf in range(2):
                            c0 = qh * 1024 + hf * 512
                            c1 = min(c0 + 512, lastc)
                            if c1 <= c0:
                                continue
                            nc.tensor.matmul(
                                out=ptk[qh][:, hf * 512 : hf * 512 + (c1 - c0)],
                                lhsT=w_sb["wk"][:, dc, :],
                                rhs=xc[:, c0 : c1],
                                start=(dc == 0),
                                stop=(dc == ND - 1),
                            )
                nc.scalar.copy(out=kT_sb[:, b, 0:1024], in_=ptk[0][:])
                nc.vector.tensor_copy(out=kT_sb[:, b, 1024:2048], in_=ptk[1][:])
                # --- q projection -> qT_sb (+ bq) ---
                ptq = [psS.tile([128, 1024], f32, tag="mm", name=f"pq{b}{i}")
                       for i in range(2)]
                for dc in range(ND):
                    xc = xt.tile([128, S], bf16, tag="xc", name="xcq")
                    nc.sync.dma_start(
                        out=xc[:], in_=d["queryT"][b, dc * 128 : (dc + 1) * 128, :]
                    )
                    for qh in range(2):
                        for hf in range(2):
                            nc.tensor.matmul(
                                out=ptq[qh][:, hf * 512 : (hf + 1) * 512],
                                lhsT=w_sb["wq"][:, dc, :],
                                rhs=xc[:, qh * 1024 + hf * 512 : qh * 1024 + (hf + 1) * 512],
                                start=(dc == 0),
                                stop=(dc == ND - 1),
                            )
                nc.scalar.add(out=qT_sb[:, b, 0:1024], in_=ptq[0][:], add=bq_sb[:])
                nc.vector.tensor_scalar_add(
                    out=qT_sb[:, b, 1024:2048], in0=ptq[1][:], scalar1=bq_sb[:]
                )

            def proj_v(b):
                # --- v projection, direct [s, dh] layout ---
                psv = [psT.tile([128, 1024], f32, tag="at", name=f"pv{b}{i}")
                       for i in range(2)]
                for dc in range(ND):
                    xc = xt.tile([128, S], bf16, tag="xc", name="xcv")
                    nc.sync.dma_start(
                        out=xc[:], in_=d["valueT"][b, dc * 128 : (dc + 1) * 128, :]
                    )
                    for st in range(NK):
                        # start_tensor_calc zeroes the whole 2KB PSUM bank (4
                        # st-regions): only the bank-first st may set it. Skip
                        # fully-masked st tiles unless needed for bank zeroing.
                        if (b, st) in full_tiles and (
                            st % 4 != 0
                            or all((b, s) in full_tiles
                                   for s in range(st, min(st + 4, NK)))
                        ):
                            continue
                        nc.tensor.matmul(
                            out=psv[st // 8][:, (st % 8) * 128 : (st % 8 + 1) * 128],
                            lhsT=xc[:, st * 128 : (st + 1) * 128],
                            rhs=w_sb["wv"][:, dc, :],
                            start=(dc == 0 and st % 4 == 0),
                            stop=(dc == ND - 1),
                            skip_group_check=True,
                        )
                # copy into v_sb (bf16), zeroing masked key rows where needed
                for half in range(2):
                    sts = [st for st in range(half * 8, (half + 1) * 8)]
                    simple = [st for st in sts
                              if (b, st) not in part_tiles and (b, st) not in full_tiles]
                    # bulk-copy the longest contiguous prefix run of simple tiles
                    run = []
                    for st in sts:
                        if st in simple and (not run or st == run[-1] + 1):
                            run.append(st)
                        elif not run:
                            continue
                        else:
                            break
                    if run:
                        st0, n = run[0], len(run)
                        nc.vector.tensor_copy(
                            out=v_sb[:, b, st0 : st0 + n, :, 0:64],
                            in_=psv[half][
                                :, (st0 - half * 8) * 128 : (st0 - half * 8 + n) * 128
                            ].rearrange("p (t h m) -> p t h m", t=n, h=HPC),
                        )
                    for st in sts:
                        if st in run or (b, st) in full_tiles:
                            continue
                        i0 = (st - half * 8) * 128
                        if (b, st) in part_tiles:
                            nc.vector.tensor_scalar_mul(
                                out=v_sb[:, b, st, :, 0:64],
                                in0=psv[half][:, i0 : i0 + 128].rearrange(
                                    "p (h m) -> p h m", h=HPC
                                ),
                                scalar1=vm_sb[:, b, st : st + 1],
                            )
                        else:
                            nc.vector.tensor_copy(
                                out=v_sb[:, b, st, :, 0:64],
                                in_=psv[half][:, i0 : i0 + 128].rearrange(
                                    "p (h m) -> p h m", h=HPC
                                ),
                            )
            # ------- Phase 2: attention + fused norm; o-proj interleaved -------
            def oproj_chunk(qq, b, do, pool=None):
                q0 = qq * 1024

                def emit():
                    po = (pool or psS).tile([128, 1024], f32,
                                            tag="mm" if (pool or psS) is psS else "at",
                                            name="po")
                    for hf in range(2):
                        hs512 = slice(hf * 512, (hf + 1) * 512)
                        nc.tensor.matmul(
                            out=po[:, hs512],
                            lhsT=wo_sb[:, do * 128 : (do + 1) * 128],
                            rhs=au_sb[
                                :, b, q0 + hf * 512 : q0 + (hf + 1) * 512
                            ],
                            start=True,
                            stop=True,
                        )
                    ot = otp.tile([128, 1024], bf16, tag="ot", name="ot")
                    if do % 2:
                        nc.scalar.copy(out=ot[:], in_=po[:])
                    else:
                        nc.vector.tensor_copy(out=ot[:], in_=po[:])
                    nc.sync.dma_start(
                        out=oT[b, do * 128 : (do + 1) * 128, q0 : q0 + 1024],
                        in_=ot[:],
                    )
                return emit

            def emit_block(qq, h, ochunks):
                """kk loop for (qq, h); pops one deferred o-proj chunk per kk."""
                q0 = qq * 1024
                at = [psT.tile([128, 1024], f32, tag="at", name=f"at{_i}")
                      for _i in range(B)]
                for kk in range(NK):
                    live = [b for b in range(B) if (b, kk) not in full_tiles]
                    if not live:
                        if ochunks:
                            ochunks.pop(0)()
                        continue
                    bt = btp.tile([128, 1024], bf16, tag="bt", name="bt")
                    nc.sync.dma_start(
                        out=bt[:],
                        in_=d["ebiasT"][h, kk * 128 : (kk + 1) * 128, q0 : q0 + 1024],
                    )
                    for b in live:
                        sc = psS.tile([128, 1024], f32, tag="mm", name="sc")
                        for hf in range(2):
                            hs512 = slice(hf * 512, (hf + 1) * 512)
                            nc.tensor.matmul(
                                out=sc[:, hs512],
                                lhsT=kT_sb[
                                    h * 64 : (h + 1) * 64, b, kk * 128 : (kk + 1) * 128
                                ],
                                rhs=qT_sb[
                                    h * 64 : (h + 1) * 64, b,
                                    q0 + hf * 512 : q0 + (hf + 1) * 512
                                ],
                                start=True, stop=True,
                            )
                        et = etp.tile([128, 1024], bf16, tag="et", name="et")
                        nc.scalar.activation(out=et[:], in_=sc[:], func=Exp)
                        pt = ptw.tile([128, 1024], bf16, tag="ptw", name="pt")
                        nc.vector.tensor_mul(out=pt[:], in0=et[:], in1=bt[:])
                        for hf in range(2):
                            hs512 = slice(hf * 512, (hf + 1) * 512)
                            nc.tensor.matmul(
                                out=at[b][0:65, hs512],
                                lhsT=v_sb[:, b, kk, h, 0:65],
                                rhs=pt[:, hs512],
                                start=(kk == live_kk[b][0]),
                                stop=(kk == live_kk[b][-1]),
                            )
                    if ochunks:
                        ochunks.pop(0)()
                # normalize: recip of denom row, broadcast, multiply -> au_sb
                for b in range(B):
                    rr = rrp.tile([1, 1024], f32, tag="rr", name="rr")
                    nc.scalar.activation(out=rr[:], in_=at[b][64:65, :], func=Ln)
                    nc.scalar.activation(out=rr[:], in_=rr[:], func=Exp, scale=-1.0)
                    bcs = bcp.tile([64, 1024], f32, tag="bcs", name="bcs")
                    nc.gpsimd.partition_broadcast(bcs[:], rr[:])
                    nc.vector.tensor_mul(
                        out=au_sb[h * 64 : (h + 1) * 64, b, q0 : q0 + 1024],
                        in0=at[b][0:64, :],
                        in1=bcs[:],
                    )
                while ochunks:
                    ochunks.pop(0)()

            def load_resident_bias(qq, h, btres, eng):
                """Dispatch all bias-tile DMAs for (qq, h) into btres upfront
                on the given engine queue (ACT during idle front, SP later)."""
                q0 = qq * 1024
                for kk in range(NK):
                    if any((bb, kk) not in full_tiles for bb in range(B)):
                        eng.dma_start(
                            out=btres[:, kk, :],
                            in_=d["ebiasT"][h, kk * 128 : (kk + 1) * 128,
                                            q0 : q0 + 1024],
                        )

            def attn_pass(qq, h, b, btres, ochunks=(), mid_emit=None,
                          defer=False):
                """Single-batch kk pass for (qq, h); bias read from the
                resident array btres. With bias_jit, each bias tile's DMA is
                dispatched from the ACT hwdge queue a few iterations ahead.
                The P tiles for all kk are buffered so the av matmuls trail
                the score/exp stream (avoids in-order PE stalls on v
                availability). Normalizes at the end."""
                q0 = qq * 1024
                at = psT.tile([128, 1024], f32, tag="at", name=f"at{qq}{h}{b}")
                ochunks = list(ochunks)
                bts = {}
                pts = {}
                for i, kk in enumerate(live_kk[b]):
                    if btres is None:
                        bt = btp.tile([128, 1024], bf16, tag="bt", name="bt")
                        nc.sync.dma_start(
                            out=bt[:],
                            in_=d["ebiasT"][h, kk * 128 : (kk + 1) * 128,
                                            q0 : q0 + 1024],
                        )
                        bts[kk] = bt
                    sc = psS.tile([128, 1024], f32, tag="mm", name="sc")
                    for hf in range(2):
                        hs512 = slice(hf * 512, (hf + 1) * 512)
                        nc.tensor.matmul(
                            out=sc[:, hs512],
                            lhsT=kT_sb[
                                h * 64 : (h + 1) * 64, b, kk * 128 : (kk + 1) * 128
                            ],
                            rhs=qT_sb[
                                h * 64 : (h + 1) * 64, b,
                                q0 + hf * 512 : q0 + (hf + 1) * 512
                            ],
                            start=True, stop=True,
                        )
                    et = etp.tile([128, 1024], bf16, tag="et", name="et")
                    nc.scalar.activation(out=et[:], in_=sc[:], func=Exp)
                    pt = ptw.tile([128, 1024], bf16, tag="ptw", name="ptw")
                    src_bt = bts[kk][:] if btres is None else btres[:, kk, :]
                    nc.vector.tensor_mul(out=pt[:], in0=et[:], in1=src_bt)
                    pts[kk] = pt
                def finish():
                    if mid_emit is not None:
                        mid_emit()
                    for kk in live_kk[b]:
                        for hf in range(2):
                            hs512 = slice(hf * 512, (hf + 1) * 512)
                            nc.tensor.matmul(
                                out=at[0:65, hs512],
                                lhsT=v_sb[:, b, kk, h, 0:65],
                                rhs=pts[kk][:, hs512],
                                start=(kk == live_kk[b][0]),
                                stop=(kk == live_kk[b][-1]),
                            )
                        if ochunks:
                            ochunks.pop(0)()
                    # normalize: recip of denom row, broadcast, multiply
                    dn = rrp.tile([1, 1024], f32, tag="dn", name="dn")
                    nc.vector.tensor_copy(out=dn[:], in_=at[64:65, :])
                    rr = rrp.tile([1, 1024], f32, tag="rr", name="rr")
                    nc.vector.reciprocal_approx_fast(out=rr[:], in_=dn[:])
                    bcs = bcp.tile([64, 1024], f32, tag="bcs", name="bcs")
                    nc.gpsimd.partition_broadcast(bcs[:], rr[:])
                    nc.vector.tensor_mul(
                        out=au_sb[h * 64 : (h + 1) * 64, b, q0 : q0 + 1024],
                        in0=at[0:64, :],
                        in1=bcs[:],
                    )
                    for f in ochunks:
                        f()
                if defer:
                    return finish
                finish()

            proj_kq(0)
            nc.sync.dma_start(out=vm_sb[:], in_=d["vmask"][:])
            for h in range(HPC):
                nc.sync.dma_start(out=v_sb[:, :, :, h, 64:65], in_=d["vcol"][:])
            load_resident_bias(0, 0, btresA, nc.sync)
            # scores(0,0,0) stream first; v(0) projection + avs trail it
            attn_pass(0, 0, 0, btresA, mid_emit=lambda: proj_v(0))
            load_resident_bias(0, 1, btresB, nc.sync)
            attn_pass(0, 1, 0, btresB)
            proj_kq(1)
            # batch-0 pass of (1,0) fills the b1-projection DMA window:
            # its exps have no new deps; bias arrives later for the mults.
            # Its avs are deferred past C's score stream so the late bias
            # doesn't block the in-order PE queue.
            g_fin = attn_pass(1, 0, 0, None, defer=True)
            nc.sync.dma_start(out=wo_sb[:], in_=d["woT"][:])
            c_fin = attn_pass(0, 0, 1, btresA, defer=True)
            g_fin()
            proj_v(1)
            c_fin()
            attn_pass(0, 1, 1, btresB)
            load_resident_bias(1, 0, btresB, nc.sync)
            attn_pass(1, 0, 1, btresB,
                      ochunks=[oproj_chunk(0, b, do)
                               for do in range(ND) for b in range(B)])
            load_resident_bias(1, 1, btresA, nc.sync)
            attn_pass(1, 1, 0, btresA)
            attn_pass(1, 1, 1, btresA,
                      ochunks=[oproj_chunk(1, 0, do) for do in range(ND)])
            for do in range(ND):
                oproj_chunk(1, 1, do, pool=psT if do % 2 else psS)()
    if not nc.is_finalized():
        nc.finalize()
    return nc


def _mask_key(mask):
    """Classify (b, kk) tiles: 'full' = all masked out, 'part' = partially."""
    full, part = set(), set()
    for b in range(B):
        m = mask[b].reshape(NK, 128)
        for kk in range(NK):
            n = int(m[kk].sum())
            if n == 0:
                full.add((b, kk))
            elif n < 128:
                part.add((b, kk))
    return frozenset(full), frozenset(part)


def kernel(query, key, value, key_padding_mask, relative_bias,
           Wq, bq, Wk, bk, Wv, bv, Wo, bo, **_unused):
    query = np.asarray(query, dtype=np.float32)
    key = np.asarray(key, dtype=np.float32)
    value = np.asarray(value, dtype=np.float32)
    mask = np.asarray(key_padding_mask)
    relative_bias = np.asarray(relative_bias, dtype=np.float32)
    Wq, bq = np.asarray(Wq, np.float32), np.asarray(bq, np.float32)
    Wk = np.asarray(Wk, np.float32)
    Wv, bv = np.asarray(Wv, np.float32), np.asarray(bv, np.float32)
    Wo, bo = np.asarray(Wo, np.float32), np.asarray(bo, np.float32)

    queryT = np.ascontiguousarray(query.transpose(0, 2, 1)).astype(ml_dtypes.bfloat16)
    keyT = np.ascontiguousarray(key.transpose(0, 2, 1)).astype(ml_dtypes.bfloat16)
    valueT = np.ascontiguousarray(value.transpose(0, 2, 1)).astype(ml_dtypes.bfloat16)
    maskf = mask.astype(np.float32)  # (B, S) 1.0 live / 0.0 masked
    vmask = np.ascontiguousarray(
        maskf.reshape(B, NK, 128).transpose(2, 0, 1)
    )  # (128, B, NK)
    vcol = vmask[:, :, :, None].astype(ml_dtypes.bfloat16)  # (128, B, NK, 1)
    ebiasT = np.exp(
        relative_bias[0].transpose(0, 2, 1)
    ).astype(ml_dtypes.bfloat16)  # (H, S, S) keys-major
    sc = 1.0 / np.sqrt(DH)
    # bv's effect: softmax rows sum to 1 -> out += Wo @ bv (host); bk cancels.
    bo_eff = bo + Wo @ bv

    in_maps = []
    for c in range(NC):
        hs = slice(c * HPC * DH, (c + 1) * HPC * DH)  # this core's 128 head rows
        in_maps.append({
            "queryT": queryT, "keyT": keyT, "valueT": valueT,
            "ebiasT": np.ascontiguousarray(ebiasT[c * HPC : (c + 1) * HPC]),
            "vcol": vcol, "vmask": vmask,
            "wqT": np.ascontiguousarray(((Wq[hs] * sc).T).reshape(8, 128, 128).transpose(1, 0, 2).reshape(128, -1)).astype(ml_dtypes.bfloat16),
            "wkT": np.ascontiguousarray((Wk[hs].T).reshape(8, 128, 128).transpose(1, 0, 2).reshape(128, -1)).astype(ml_dtypes.bfloat16),
            "wvT": np.ascontiguousarray((Wv[hs].T).reshape(8, 128, 128).transpose(1, 0, 2).reshape(128, -1)).astype(ml_dtypes.bfloat16),
            "bq": (bq[hs] * sc).reshape(128, 1).astype(np.float32),
            "woT": np.ascontiguousarray(Wo[:, hs].T).astype(ml_dtypes.bfloat16),
        })

    global _LAST_IN_MAPS, _LAST_KEY
    _LAST_IN_MAPS = in_maps
    keyk = _mask_key(mask)
    _LAST_KEY = keyk
    if keyk not in _PROGRAMS:
        _PROGRAMS[keyk] = _build_program(*keyk)
    res = run_bass_kernel_spmd(_PROGRAMS[keyk], in_maps, list(range(NC)))
    acc = np.zeros((B, D, S), dtype=np.float32)
    for r in res.results:
        acc += r["oT"].astype(np.float32)
    return acc.transpose(0, 2, 1) + bo_eff


def run_profiled(inputs=None):
    """Timeline-simulator timing (cost-model) for the cached program, ns."""
    from concourse.timeline_sim import TimelineSim

    nc = _PROGRAMS[_LAST_KEY]
    sim = TimelineSim(nc, trace=False)
    return int(sim.simulate())

